# revision 6
# baseline (speedup 1.0000x reference)
"""Distributed Trainium2 kernel for nn_ContrastiveLoss (n=4096, d=512, 100 labels).

Math (equivalent restructuring of the reference):
  f = enc / max(||enc||, 1e-8)            row-normalized features
  sim = f @ f.T                            [n, n]
  mask_ij = (lab_i == lab_j)
  negsum_i = sum_{j: lab_j != lab_i} exp(sim_ij / T)
  loss_mat nonzeros are exactly the same-label off-diagonal pairs, where
    -log(exp(sim_ij/T) / negsum_i) = log(negsum_i) - sim_ij/T
  => numer = sum_i [ (cnt_i - 1) * log(negsum_i)
                     - (2 * possum_i - 2) ]          (T = 0.5)
     where cnt_i = |class(lab_i)|, possum_i = f_i . C[lab_i],
     C[l] = sum_{j: lab_j = l} f_j  (includes sim_ii = 1, corrected by -2)
     nnz = sum_i (cnt_i - 1)
  loss = numer / (nnz + 1e-5)

Per-core (8 cores, data-parallel over 512-row blocks):
  - every core normalizes + transposes all 4096 rows (fT, bf16) - cheaper
    than an all-gather of normalized features
  - mask is never materialized: a one-hot matmul accumulates -B*mask into
    the sim PSUM before exp, so exp(2*(sim - B*mask)) == 0 on same-label
    pairs (B=128 -> exp(-254) underflows to exactly 0)
  - negsum comes free from the exp ACT op's accum_out
  - possum via class-sum matrix C (PE matmuls, no gather)
  - per-core partial [numer_0..3 | nnz_0..3] in out[1,8]; host sums.
"""

import sys

for _p in ("/opt/trn_rl_repo", "/root/.axon_site/_ro/trn_rl_repo"):
    if _p not in sys.path:
        sys.path.append(_p)

import numpy as np

import concourse.bacc as bacc
import concourse.tile as tile
from concourse import mybir, masks
from concourse.bass_utils import run_bass_kernel_spmd

F32 = mybir.dt.float32
BF16 = mybir.dt.bfloat16
AF = mybir.ActivationFunctionType
ALU = mybir.AluOpType

N = 4096          # rows
D = 512           # feature dim
L = 100           # label values
P = 128           # partitions
N_CORES = 8
RPC = N // N_CORES          # rows per core = 512
RT = N // P                 # full row tiles = 32
RTL = RPC // P              # local row tiles = 4
KC = D // P                 # contraction chunks = 4
NB = N // 512               # column blocks of 512 = 8
BIAS = 128.0                # mask bias (exp(2*(sim-BIAS)) == 0 exactly)
GRP = 8                     # row tiles per norm batch


def build_nc(trace_friendly=False):
    nc = bacc.Bacc("TRN2", target_bir_lowering=False, debug=False,
                   num_devices=N_CORES)
    enc = nc.dram_tensor("enc", [N, D], F32, kind="ExternalInput").ap()
    encR = nc.dram_tensor("encR", [RPC, D], F32, kind="ExternalInput").ap()
    labn = nc.dram_tensor("labn", [N], BF16, kind="ExternalInput").ap()
    labR = nc.dram_tensor("labR", [RPC], BF16, kind="ExternalInput").ap()
    labg = nc.dram_tensor("labg", [RT, P], BF16, kind="ExternalInput").ap()
    iota_b = nc.dram_tensor("iota_b", [L], BF16, kind="ExternalInput").ap()
    iota_f = nc.dram_tensor("iota_f", [L], F32, kind="ExternalInput").ap()
    out = nc.dram_tensor("out", [1, 8], F32, kind="ExternalOutput").ap()

    with tile.TileContext(nc) as tc:
        _build_body(nc, tc, enc, encR, labn, labR, labg, iota_b, iota_f, out)
    nc.compile()
    return nc


def _build_body(nc, tc, enc, encR, labn, labR, labg, iota_b, iota_f, out):
    from contextlib import ExitStack
    with ExitStack() as ctx:
        consts = ctx.enter_context(tc.tile_pool(name="consts", bufs=1))
        big = ctx.enter_context(tc.tile_pool(name="big", bufs=1))
        work = ctx.enter_context(tc.tile_pool(name="work", bufs=12))
        fpool = ctx.enter_context(tc.tile_pool(name="fpool", bufs=10))
        ohp = ctx.enter_context(tc.tile_pool(name="ohp", bufs=4))
        sm = ctx.enter_context(tc.tile_pool(name="sm", bufs=1))
        ps_tr = ctx.enter_context(tc.tile_pool(name="ps_tr", bufs=4, space="PSUM"))
        ps_c = ctx.enter_context(tc.tile_pool(name="ps_c", bufs=1, space="PSUM"))
        ps_sim = ctx.enter_context(tc.tile_pool(name="ps_sim", bufs=2, space="PSUM"))
        ps_aux = ctx.enter_context(tc.tile_pool(name="ps_aux", bufs=1, space="PSUM"))

        # ---------------- constants ----------------
        ones_bf = consts.tile([1, P], BF16)
        nc.vector.memset(ones_bf[:], 1.0)
        ident = consts.tile([P, P], BF16)
        masks.make_identity(nc, ident[:])
        iota_r = consts.tile([1, L], BF16)
        nc.sync.dma_start(out=iota_r[:], in_=iota_b[None, :])
        iota_c = consts.tile([L, 1], F32)
        nc.sync.dma_start(out=iota_c[:], in_=iota_f[:, None])
        labn_t = consts.tile([1, N], BF16)
        nc.sync.dma_start(out=labn_t[:], in_=labn[None, :])
        labR_t = consts.tile([1, RPC], BF16)
        nc.sync.dma_start(out=labR_t[:], in_=labR[None, :])
        labg_t = consts.tile([RT, P], BF16)
        nc.sync.dma_start(out=labg_t[:], in_=labg[:])

        junk1 = consts.tile([P, 1], F32)

        # iota_bcast[p, l] = l  (for row-major one-hot build)
        ps_a = ps_aux.tile([P, L], F32, tag="aux")
        nc.tensor.matmul(ps_a[:], lhsT=ones_bf[:], rhs=iota_r[:],
                         start=True, stop=True)
        iota_bc = consts.tile([P, L], F32)
        nc.vector.tensor_copy(out=iota_bc[:], in_=ps_a[:])

        # lab_part_all[p, t] = label of row t*128+p  (transpose of labg)
        ps_a = ps_aux.tile([P, RT], BF16, tag="aux")
        nc.tensor.transpose(ps_a[:], labg_t[:], ident[:RT, :RT])
        lab_part = consts.tile([P, RT], F32)
        nc.vector.tensor_copy(out=lab_part[:], in_=ps_a[:])

        # onehotT[l, j] = (lab_j == l) over all n columns, and -B-scaled copy
        ohT = big.tile([L, N], BF16)
        ohTB = big.tile([L, N], BF16)
        cc_stage = sm.tile([L, NB], F32)
        for b in range(NB):
            ps_a = ps_aux.tile([L, 512], F32, tag="aux")
            nc.tensor.matmul(ps_a[:], lhsT=ones_bf[:, :L],
                             rhs=labn_t[:, b * 512:(b + 1) * 512],
                             start=True, stop=True)
            nc.vector.tensor_scalar(
                out=ohT[:, b * 512:(b + 1) * 512], in0=ps_a[:],
                scalar1=iota_c[:], scalar2=0.0, op0=ALU.is_equal,
                op1=ALU.add, accum_out=cc_stage[:, b:b + 1])
            nc.vector.tensor_scalar(
                out=ohTB[:, b * 512:(b + 1) * 512],
                in0=ohT[:, b * 512:(b + 1) * 512],
                scalar1=-BIAS, scalar2=None, op0=ALU.mult)
        # class counts
        cc = sm.tile([L, 1], F32)
        nc.vector.reduce_sum(out=cc[:], in_=cc_stage[:], axis=mybir.AxisListType.X)
        cc_bf = sm.tile([L, 1], BF16)
        nc.vector.tensor_copy(out=cc_bf[:], in_=cc[:])

        # onehotT_R[l, m] for this core's rows (lhsT of bias/Cg/cnt matmuls)
        ps_a = ps_aux.tile([L, RPC], F32, tag="aux")
        nc.tensor.matmul(ps_a[:], lhsT=ones_bf[:, :L], rhs=labR_t[:],
                         start=True, stop=True)
        ohR = big.tile([L, RPC], BF16)
        nc.vector.tensor_scalar(out=ohR[:], in0=ps_a[:], scalar1=iota_c[:],
                                scalar2=None, op0=ALU.is_equal)

        # ---------------- local rows: normalize + transpose ----------------
        encR_t = [None] * RTL
        nsqL = sm.tile([P, RTL], F32)
        for t in range(RTL):
            encR_t[t] = fpool.tile([P, D], F32, tag="encR", name=f"encR_t{t}")
            nc.sync.dma_start(out=encR_t[t][:], in_=encR[t * P:(t + 1) * P, :])
            nc.scalar.activation(out=junk1.broadcast_to([P, D]), in_=encR_t[t][:],
                                 func=AF.Square, accum_out=nsqL[:, t:t + 1])
        invL = sm.tile([P, RTL], F32)
        nc.scalar.activation(out=invL[:], in_=nsqL[:], func=AF.Sqrt)
        nc.vector.tensor_scalar(out=invL[:], in0=invL[:], scalar1=1e-8,
                                scalar2=None, op0=ALU.max)
        nc.vector.reciprocal(invL[:], invL[:])

        f_loc = [None] * RTL
        for t in range(RTL):
            f_loc[t] = big.tile([P, D], BF16, tag=f"floc{t}", name=f"f_loc{t}")
            nc.vector.tensor_scalar(out=f_loc[t][:], in0=encR_t[t][:],
                                    scalar1=invL[:, t:t + 1], scalar2=None,
                                    op0=ALU.mult)
        fT_loc = [None] * KC
        for k in range(KC):
            ps_t = ps_tr.tile([P, RPC], BF16, tag="ps_tr")
            for t in range(RTL):
                nc.tensor.transpose(ps_t[:, t * P:(t + 1) * P],
                                    f_loc[t][:, k * P:(k + 1) * P], ident[:])
            fT_loc[k] = big.tile([P, RPC], BF16, tag=f"fTloc{k}", name=f"fT_loc{k}")
            nc.vector.tensor_copy(out=fT_loc[k][:], in_=ps_t[:])

        # ---------------- all rows: normalize + transpose + class sums -----
        fT = [big.tile([P, N], BF16, tag=f"fT{k}", name=f"fT{k}") for k in range(KC)]
        ps_cm = ps_c.tile([L, D], F32)
        nsq = sm.tile([P, RT], F32)
        inv = sm.tile([P, RT], F32)

        n_groups = RT // GRP
        for g in range(n_groups):
            t0 = g * GRP
            enc_t = [None] * GRP
            for i in range(GRP):
                t = t0 + i
                enc_t[i] = work.tile([P, D], F32, tag="enc", name=f"enc_t{t}")
                nc.sync.dma_start(out=enc_t[i][:], in_=enc[t * P:(t + 1) * P, :])
                nc.scalar.activation(out=junk1.broadcast_to([P, D]),
                                     in_=enc_t[i][:], func=AF.Square,
                                     accum_out=nsq[:, t:t + 1])
            gs = slice(t0, t0 + GRP)
            nc.scalar.activation(out=inv[:, gs], in_=nsq[:, gs], func=AF.Sqrt)
            nc.vector.tensor_scalar(out=inv[:, gs], in0=inv[:, gs], scalar1=1e-8,
                                    scalar2=None, op0=ALU.max)
            nc.vector.reciprocal(inv[:, gs], inv[:, gs])

            fnorm = [None] * GRP
            for i in range(GRP):
                t = t0 + i
                fnorm[i] = fpool.tile([P, D], BF16, tag="fnorm", name=f"fnorm{t}")
                nc.vector.tensor_scalar(out=fnorm[i][:], in0=enc_t[i][:],
                                        scalar1=inv[:, t:t + 1], scalar2=None,
                                        op0=ALU.mult)
                # row-major one-hot for class-sum matmul
                oh_row = ohp.tile([P, L], BF16, tag="ohrow")
                nc.vector.tensor_scalar(out=oh_row[:], in0=iota_bc[:],
                                        scalar1=lab_part[:, t:t + 1],
                                        scalar2=None, op0=ALU.is_equal)
                nc.tensor.matmul(ps_cm[:], lhsT=oh_row[:], rhs=fnorm[i][:],
                                 start=(t == 0), stop=(t == RT - 1))
            # transposes: for each k-chunk, pack 4 row tiles per psum tile
            for half in range(GRP // 4):
                ts = [t0 + half * 4 + j for j in range(4)]
                for k in range(KC):
                    ps_t = ps_tr.tile([P, 512], BF16, tag="ps_tr")
                    for j, t in enumerate(ts):
                        nc.tensor.transpose(
                            ps_t[:, j * P:(j + 1) * P],
                            fnorm[half * 4 + j][:, k * P:(k + 1) * P], ident[:])
                    nc.vector.tensor_copy(
                        out=fT[k][:, ts[0] * P:(ts[0] + 4) * P], in_=ps_t[:])

        # class-sum matrix C -> bf16
        C_bf = big.tile([L, D], BF16)
        nc.vector.tensor_copy(out=C_bf[:], in_=ps_cm[:])

        # ---------------- main loop: sim blocks -> exp row-sums ------------
        negst = sm.tile([P, RTL * NB], F32)
        for m in range(RTL):
            for b in range(NB):
                ps_s = ps_sim.tile([P, 512], F32)
                for k in range(KC):
                    nc.tensor.matmul(ps_s[:],
                                     lhsT=fT_loc[k][:, m * P:(m + 1) * P],
                                     rhs=fT[k][:, b * 512:(b + 1) * 512],
                                     start=(k == 0), stop=False)
                nc.tensor.matmul(ps_s[:], lhsT=ohR[:, m * P:(m + 1) * P],
                                 rhs=ohTB[:, b * 512:(b + 1) * 512],
                                 start=False, stop=True)
                nc.scalar.activation(out=junk1.broadcast_to([P, 512]),
                                     in_=ps_s[:], func=AF.Exp, scale=2.0,
                                     accum_out=negst[:, m * NB + b:m * NB + b + 1])

        # ---------------- finalize ----------------
        negsum = sm.tile([P, RTL], F32)
        possum = sm.tile([P, RTL], F32)
        cnt = sm.tile([P, RTL], F32)
        for m in range(RTL):
            nc.vector.reduce_sum(out=negsum[:, m:m + 1],
                                 in_=negst[:, m * NB:(m + 1) * NB],
                                 axis=mybir.AxisListType.X)
            ps_g = ps_aux.tile([P, D], F32, tag="aux")
            nc.tensor.matmul(ps_g[:], lhsT=ohR[:, m * P:(m + 1) * P],
                             rhs=C_bf[:], start=True, stop=True)
            nc.vector.scalar_tensor_tensor(
                out=junk1.broadcast_to([P, D]), in0=f_loc[m][:], scalar=1.0,
                in1=ps_g[:], op0=ALU.mult, op1=ALU.mult,
                accum_out=possum[:, m:m + 1])
            ps_n = ps_aux.tile([P, 1], F32, tag="aux")
            nc.tensor.matmul(ps_n[:], lhsT=ohR[:, m * P:(m + 1) * P],
                             rhs=cc_bf[:], start=True, stop=True)
            nc.vector.tensor_copy(out=cnt[:, m:m + 1], in_=ps_n[:])

        logns = sm.tile([P, RTL], F32)
        nc.scalar.activation(out=logns[:], in_=negsum[:], func=AF.Ln)
        pack = sm.tile([P, 2 * RTL], BF16)
        # cnt-1 (also the per-row nonzero count)
        nc.vector.tensor_scalar(out=pack[:, RTL:], in0=cnt[:], scalar1=1.0,
                                scalar2=None, op0=ALU.subtract)
        cntm1 = sm.tile([P, RTL], F32)
        nc.vector.tensor_scalar(out=cntm1[:], in0=cnt[:], scalar1=1.0,
                                scalar2=None, op0=ALU.subtract)
        contrib = sm.tile([P, RTL], F32)
        nc.vector.tensor_tensor(out=contrib[:], in0=cntm1[:], in1=logns[:],
                                op=ALU.mult)
        pos2 = sm.tile([P, RTL], F32)
        nc.vector.tensor_scalar(out=pos2[:], in0=possum[:], scalar1=-2.0,
                                scalar2=2.0, op0=ALU.mult, op1=ALU.add)
        nc.vector.tensor_tensor(out=pack[:, :RTL], in0=contrib[:], in1=pos2[:],
                                op=ALU.add)

        ones_col = consts.tile([P, 1], BF16)
        nc.vector.memset(ones_col[:], 1.0)
        ps_f = ps_aux.tile([1, 2 * RTL], F32, tag="aux")
        nc.tensor.matmul(ps_f[:], lhsT=ones_col[:], rhs=pack[:],
                         start=True, stop=True)
        out_sb = sm.tile([1, 2 * RTL], F32)
        nc.vector.tensor_copy(out=out_sb[:], in_=ps_f[:])
        nc.sync.dma_start(out=out[:], in_=out_sb[:])


_NC_CACHE = None


def _get_nc():
    global _NC_CACHE
    if _NC_CACHE is None:
        _NC_CACHE = build_nc()
    return _NC_CACHE


def _make_in_maps(enc_features: np.ndarray, labels: np.ndarray):
    bf16 = mybir.dt.np(BF16)
    enc = np.ascontiguousarray(enc_features, dtype=np.float32)
    lab_b = labels.astype(np.float32).astype(bf16)
    labg = lab_b.reshape(RT, P)
    iota_bv = np.arange(L, dtype=np.float32).astype(bf16)
    iota_fv = np.arange(L, dtype=np.float32)
    in_maps = []
    for c in range(N_CORES):
        in_maps.append({
            "enc": enc,
            "encR": np.ascontiguousarray(enc[c * RPC:(c + 1) * RPC]),
            "labn": lab_b,
            "labR": np.ascontiguousarray(lab_b[c * RPC:(c + 1) * RPC]),
            "labg": labg,
            "iota_b": iota_bv,
            "iota_f": iota_fv,
        })
    return in_maps


def _combine(results) -> np.float32:
    # On the Neuron backend the reference's -log(1.0) is the ACT spline value
    # 6.1e-13 (not exactly 0), so count_nonzero(loss_mat) == n*n exactly: every
    # different-label and diagonal entry counts. The same entries contribute
    # only ~n^2 * 6e-13 ~ 1e-5 to the sum, which is negligible.
    numer = 0.0
    for r in results:
        o = r["out"].reshape(-1)
        numer += float(o[:RTL].sum())
    return np.float32(numer / (float(N) * float(N) + 1e-5))


def kernel(enc_features: np.ndarray, labels: np.ndarray) -> np.ndarray:
    nc = _get_nc()
    in_maps = _make_in_maps(enc_features, labels)
    res = run_bass_kernel_spmd(nc, in_maps, list(range(N_CORES)))
    return _combine(res.results)


if __name__ == "__main__":
    rng = np.random.default_rng(0)
    enc = rng.standard_normal((N, D)).astype(np.float32)
    lab = rng.integers(0, L, N).astype(np.int64)
    print("loss:", kernel(enc, lab))
